# revision 8
# baseline (speedup 1.0000x reference)
"""Trainium2 Bass kernel for nn_KinematicOperation — v3: fp16 planar DVE.

Same blocked-scan algorithm as v1, but the lane-parallel phases (bond fold,
level-1 scan, w, cumsum, down-transform) run in fp16 with entry-PLANAR
layouts (inner dim = lanes, unit stride, 4B aligned), which engages the
DVE's 2x_1P mode: tensor_tensor at 2 elem/cycle.  Angles stay fp32 through
the ACT sine (fp16 angle rounding would dominate the error budget); sin/cos
outputs and all downstream per-atom products are fp16.  The block-level
hierarchy (bht, levels, excl) stays fp32 packed as in v1.  ACT does the
transposing casts (trig planes, d-column, rx expansion, output repack).

Layouts per partition (generation with F lanes, T slabs):
  X[t][e][f]   e in 0..8: 3x3 entries row-major; slabs t (fp16)
  w[t][c][f]   c in 0..2 (fp16)
  trig planes nm[t*F + f] (fp16), angles atom-major fp32
  bht/lp2/spx/rx packed 12-elem HTs per lane (fp32), rx16 planar fp16
"""

import os
import sys

import numpy as np

for _p in ("/opt/trn_rl_repo", "/root/.axon_site/_ro/trn_rl_repo"):
    if os.path.isdir(_p) and _p not in sys.path:
        sys.path.insert(0, _p)

C0, L0 = 2048, 768
C1, L1 = 2048, 256
N = 1 + C0 * L0 + C1 * L1
BOFF = 1 + C0 * L0
NCORES = 8
P = 128
CHI = 2
CH0 = C0 // NCORES
CH1 = C1 // NCORES
A0 = CH0 * L0
A1 = CH1 * L1

T0, J0, S0, U0 = 12, 64, 8, 8
F0 = CHI * J0
T1, J1, S1, U1 = 8, 32, 4, 8
F1 = CHI * J1

PI = float(np.pi)

_CACHE = {}


def _build_program(repeat=1):
    from concourse import bacc, mybir, tile
    from concourse.bass import AP

    f32 = mybir.dt.float32
    f16 = mybir.dt.float16
    SIN = mybir.ActivationFunctionType.Sin
    ABS = mybir.ActivationFunctionType.Abs
    CPY = mybir.ActivationFunctionType.Copy

    nc = bacc.Bacc("TRN2", target_bir_lowering=False, debug=False)

    th0_d = nc.dram_tensor("th0", [P, CHI * L0], f32, kind="ExternalInput")
    al0_d = nc.dram_tensor("al0", [P, CHI * L0], f32, kind="ExternalInput")
    dt0_d = nc.dram_tensor("dt0", [P, CHI * L0], f32, kind="ExternalInput")
    th1_d = nc.dram_tensor("th1", [P, CHI * L1], f32, kind="ExternalInput")
    al1_d = nc.dram_tensor("al1", [P, CHI * L1], f32, kind="ExternalInput")
    dt1_d = nc.dram_tensor("dt1", [P, CHI * L1], f32, kind="ExternalInput")
    jd_d = nc.dram_tensor("jd", [P, CHI * 9], f32, kind="ExternalInput")
    kin0_d = nc.dram_tensor("kin0", [P, F0 * T0 * 3], f16,
                            kind="ExternalOutput")
    kin1_d = nc.dram_tensor("kin1", [P, F1 * T1 * 3], f16,
                            kind="ExternalOutput")

    def apx(tl, off, *dims):
        t = tl[:] if not isinstance(tl, AP) else tl
        return AP(t.tensor, t.offset + off,
                  [[t.ap[0][0], P]] + [list(d) for d in dims])

    def compose_1d(E, lanes, a_off, a_step, b_off, b_step, o_off, o_step,
                   tA, tB, a_tile, b_tile, o_tile):
        for k, dst in ((0, tA), (1, tB)):
            E.tensor_mul(
                out=apx(dst, 0, (12, lanes), (4, 3), (1, 4)),
                in0=apx(a_tile, a_off + k, (a_step, lanes), (4, 3), (0, 4)),
                in1=apx(b_tile, b_off + 4 * k, (b_step, lanes), (0, 3), (1, 4)),
            )
        E.tensor_add(
            out=apx(tA, 0, (12, lanes), (1, 12)),
            in0=apx(tA, 0, (12, lanes), (1, 12)),
            in1=apx(tB, 0, (12, lanes), (1, 12)))
        E.tensor_mul(
            out=apx(tB, 0, (12, lanes), (4, 3), (1, 4)),
            in0=apx(a_tile, a_off + 2, (a_step, lanes), (4, 3), (0, 4)),
            in1=apx(b_tile, b_off + 8, (b_step, lanes), (0, 3), (1, 4)),
        )
        E.tensor_add(
            out=apx(o_tile, o_off, (o_step, lanes), (1, 12)),
            in0=apx(tA, 0, (12, lanes), (1, 12)),
            in1=apx(tB, 0, (12, lanes), (1, 12)),
        )
        E.tensor_add(
            out=apx(o_tile, o_off + 3, (o_step, lanes), (4, 3)),
            in0=apx(o_tile, o_off + 3, (o_step, lanes), (4, 3)),
            in1=apx(a_tile, a_off + 3, (a_step, lanes), (4, 3)),
        )

    def excl_blocks(E, SC, CS, U, LPS, spx, lp2, rx, tA, tB):
        SC.copy(out=apx(rx, 0, (U * 12, CS), (1, 12)),
                in_=apx(spx, 0, (12, CS), (1, 12)))
        UM = U - 1
        # all 6 k0/k1 muls are independent; then the adds; then the k2 muls;
        # then the final adds — maximizes dep distance on the DVE
        for i in range(3):
            for k, dst in ((0, tA), (1, tB)):
                E.tensor_mul(
                    out=apx(dst, 4 * i, (96, CS), (12, UM), (1, 4)),
                    in0=apx(spx, 4 * i + k, (12, CS), (0, UM), (0, 4)),
                    in1=apx(lp2, 12 + 4 * k, (LPS, CS), (12, UM), (1, 4)))
        for i in range(3):
            E.tensor_add(
                out=apx(tA, 4 * i, (96, CS), (12, UM), (1, 4)),
                in0=apx(tA, 4 * i, (96, CS), (12, UM), (1, 4)),
                in1=apx(tB, 4 * i, (96, CS), (12, UM), (1, 4)))
        for i in range(3):
            E.tensor_mul(
                out=apx(tB, 4 * i, (96, CS), (12, UM), (1, 4)),
                in0=apx(spx, 4 * i + 2, (12, CS), (0, UM), (0, 4)),
                in1=apx(lp2, 12 + 8, (LPS, CS), (12, UM), (1, 4)))
        for i in range(3):
            E.tensor_add(
                out=apx(rx, 12 + 4 * i, (96, CS), (12, UM), (1, 4)),
                in0=apx(tA, 4 * i, (96, CS), (12, UM), (1, 4)),
                in1=apx(tB, 4 * i, (96, CS), (12, UM), (1, 4)))
        E.tensor_add(
            out=apx(rx, 12 + 3, (96, CS), (12, UM), (4, 3)),
            in0=apx(rx, 12 + 3, (96, CS), (12, UM), (4, 3)),
            in1=apx(spx, 3, (12, CS), (0, UM), (4, 3)))

    with tile.TileContext(nc) as tc:
      for _rep in range(repeat):
        with tc.tile_pool(name="main", bufs=1) as mp:
            X0 = mp.tile([P, T0 * F0 * 9], f16)
            w0 = mp.tile([P, T0 * F0 * 3], f16)
            tW1_0 = mp.tile([P, T0 * F0], f16)
            tW2_0 = mp.tile([P, T0 * F0], f16)
            tA0 = mp.tile([P, 4 * F0], f16)
            tB0 = mp.tile([P, 6 * F0], f16)
            tC0 = mp.tile([P, 6 * F0], f16)
            tg0 = {nm: mp.tile([P, T0 * F0], f16, name=f"tg0_{nm}")
                   for nm in ("sa", "ca", "st", "ct")}
            apl0 = mp.tile([P, CHI * L0], f32)
            wsc0 = mp.tile([P, CHI * L0], f32)
            aw0 = mp.tile([P, CHI * L0], f32)
            d16_0 = mp.tile([P, T0 * F0], f16)
            tg1 = {nm: mp.tile([P, T1 * F1], f16, name=f"tg1_{nm}")
                   for nm in ("sa", "ca", "st", "ct")}
            apl1 = mp.tile([P, CHI * L1], f32)
            wsc1 = mp.tile([P, CHI * L1], f32)
            aw1 = mp.tile([P, CHI * L1], f32)
            d16_1 = mp.tile([P, T1 * F1], f16)
            bht0 = mp.tile([P, F0 * 12], f16)
            rx0 = mp.tile([P, F0 * 12], f16)
            rx16_0 = mp.tile([P, F0 * 12], f16)
            lp2_0 = mp.tile([P, CHI * S0 * (U0 + 1) * 12], f16)
            spx0 = mp.tile([P, CHI * S0 * 12], f16)
            lp2_1 = mp.tile([P, CHI * S1 * (U1 + 1) * 12], f16)
            spx1 = mp.tile([P, CHI * S1 * 12], f16)
            tAh = mp.tile([P, 96 * CHI * S0], f16)
            tBh = mp.tile([P, 96 * CHI * S0], f16)
            a32t = mp.tile([P, CHI * 12], f16)
            rbr = mp.tile([P, CHI * 12], f16)
            jd = mp.tile([P, CHI * 9], f32)
            jang = mp.tile([P, CHI * 2 * 3], f32)
            jsin = mp.tile([P, CHI * 2 * 3], f32)
            jcos = mp.tile([P, CHI * 2 * 3], f32)
            re_ = mp.tile([P, CHI * 2 * 9], f32)
            rj = mp.tile([P, CHI * 9], f32)
            jtmp = mp.tile([P, CHI * 2 * 9], f32)
            halfpi = mp.tile([P, 1], f32)

            V = nc.vector
            SC = nc.scalar

            nc.sync.dma_start(out=jd[:], in_=jd_d[:])
            V.memset(halfpi[:], PI / 2)

            # prefill identities (fp32 hierarchy tiles)
            V.memset(lp2_0[:], 0.0)
            V.memset(apx(lp2_0, 0, ((U0 + 1) * 12, CHI * S0), (5, 3)), 1.0)
            V.memset(spx0[:], 0.0)
            V.memset(apx(spx0, 0, (S0 * 12, CHI), (5, 3)), 1.0)
            V.memset(lp2_1[:], 0.0)
            V.memset(apx(lp2_1, 0, ((U1 + 1) * 12, CHI * S1), (5, 3)), 1.0)
            V.memset(apx(X0, 2 * F0, (1, F0)), 0.0)  # slab0 e=2 plane

            # ---- JUMP HT build (fp32, tiny) ----
            V.tensor_copy(out=jang[:], in_=apx(jd, 3, (9, CHI), (3, 2),
                                               (1, 3)))
            V.add_range_wrap(out=jsin[:], in_=jang[:], shift=0.0,
                             bound=PI, period=2 * PI)
            SC.activation(out=jsin[:], in_=jsin[:], func=SIN)
            V.add_range_wrap(out=jcos[:], in_=jang[:], shift=PI / 2,
                             bound=PI, period=2 * PI)
            SC.activation(out=jcos[:], in_=jcos[:], func=SIN)

            CR = CHI * 2

            def sc_(tl, ang):
                return apx(tl, ang, (3, CR))

            def re(e):
                return apx(re_, e, (9, CR))

            def jt1(e):
                return apx(jtmp, e, (9, CR))

            sa_ = lambda: sc_(jsin, 0)
            sb = lambda: sc_(jsin, 1)
            s_c = lambda: sc_(jsin, 2)
            ca_ = lambda: sc_(jcos, 0)
            cb = lambda: sc_(jcos, 1)
            c_c = lambda: sc_(jcos, 2)
            V.tensor_mul(out=re(0), in0=c_c(), in1=cb())
            V.tensor_mul(out=jt1(0), in0=sb(), in1=sa_())
            V.tensor_mul(out=jt1(1), in0=sb(), in1=ca_())
            V.tensor_mul(out=jt1(2), in0=c_c(), in1=jt1(0))
            V.tensor_mul(out=jt1(3), in0=s_c(), in1=ca_())
            V.tensor_sub(out=re(1), in0=jt1(2), in1=jt1(3))
            V.tensor_mul(out=jt1(2), in0=c_c(), in1=jt1(1))
            V.tensor_mul(out=jt1(3), in0=s_c(), in1=sa_())
            V.tensor_add(out=re(2), in0=jt1(2), in1=jt1(3))
            V.tensor_mul(out=re(3), in0=s_c(), in1=cb())
            V.tensor_mul(out=jt1(2), in0=s_c(), in1=jt1(0))
            V.tensor_mul(out=jt1(3), in0=c_c(), in1=ca_())
            V.tensor_add(out=re(4), in0=jt1(2), in1=jt1(3))
            V.tensor_mul(out=jt1(2), in0=s_c(), in1=jt1(1))
            V.tensor_mul(out=jt1(3), in0=c_c(), in1=sa_())
            V.tensor_sub(out=re(5), in0=jt1(2), in1=jt1(3))
            V.tensor_scalar_mul(out=re(6), in0=sb(), scalar1=-1.0)
            V.tensor_mul(out=re(7), in0=cb(), in1=sa_())
            V.tensor_mul(out=re(8), in0=cb(), in1=ca_())
            V.tensor_mul(
                out=apx(rj, 0, (9, CHI), (3, 3), (1, 3)),
                in0=apx(re_, 0, (18, CHI), (3, 3), (0, 3)),
                in1=apx(re_, 9, (18, CHI), (0, 3), (1, 3)))
            V.tensor_mul(
                out=apx(jtmp, 0, (9, CHI), (3, 3), (1, 3)),
                in0=apx(re_, 1, (18, CHI), (3, 3), (0, 3)),
                in1=apx(re_, 12, (18, CHI), (0, 3), (1, 3)))
            V.tensor_add(out=rj[:, : CHI * 9], in0=rj[:, : CHI * 9],
                         in1=jtmp[:, : CHI * 9])
            V.tensor_mul(
                out=apx(jtmp, 0, (9, CHI), (3, 3), (1, 3)),
                in0=apx(re_, 2, (18, CHI), (3, 3), (0, 3)),
                in1=apx(re_, 15, (18, CHI), (0, 3), (1, 3)))
            V.tensor_add(out=rj[:, : CHI * 9], in0=rj[:, : CHI * 9],
                         in1=jtmp[:, : CHI * 9])

            # ======== trig (both gens): t-major fp32 inputs ========
            # host pre-transposes theta/alpha/d to device order (t-major)
            # and pre-folds alpha = phi_c(parent) + phi_p (incl. the branch
            # root fold), so trig is wrap + contiguous ACT sines only.
            def emit_trig(tht, alt, tg, wsc, aw, F, T):
                V.add_range_wrap(out=wsc[:], in_=tht[:], shift=0.0,
                                 bound=PI, period=2 * PI)
                SC.activation(out=tg["st"][:], in_=wsc[:], func=SIN)
                SC.activation(out=aw[:], in_=wsc[:], func=ABS)
                SC.activation(out=tg["ct"][:], in_=aw[:], func=SIN,
                              scale=-1.0, bias=halfpi[:])
                V.add_range_wrap(out=wsc[:], in_=alt[:], shift=0.0,
                                 bound=PI, period=2 * PI)
                SC.activation(out=tg["sa"][:], in_=wsc[:], func=SIN)
                SC.activation(out=aw[:], in_=wsc[:], func=ABS)
                SC.activation(out=tg["ca"][:], in_=aw[:], func=SIN,
                              scale=-1.0, bias=halfpi[:])

            def emit_fold(X, tg, F, T):
                def tp(nm):
                    return apx(tg[nm], 0, (F, T), (1, F))

                def xo(e):
                    return apx(X, e * F, (9 * F, T), (1, F))

                SC.activation(out=xo(0), in_=tp("ct"), func=CPY, scale=-1.0)
                SC.activation(out=xo(1), in_=tp("st"), func=CPY, scale=-1.0)
                SC.activation(out=xo(5), in_=tp("sa"), func=CPY, scale=-1.0)
                SC.activation(out=xo(8), in_=tp("ca"), func=CPY)
                V.tensor_mul(out=xo(3), in0=tp("ca"), in1=tp("st"))
                V.tensor_mul(out=xo(4), in0=tp("ca"), in1=xo(0))
                V.tensor_mul(out=xo(6), in0=tp("sa"), in1=tp("st"))
                V.tensor_mul(out=xo(7), in0=tp("sa"), in1=xo(0))

            with tc.tile_pool(name="pdof", bufs=1) as pd:
                th0t = pd.tile([P, CHI * L0], f32)
                al0t = pd.tile([P, CHI * L0], f32)
                dt0t = pd.tile([P, CHI * L0], f32)
                th1t = pd.tile([P, CHI * L1], f32)
                al1t = pd.tile([P, CHI * L1], f32)
                dt1t = pd.tile([P, CHI * L1], f32)

                nc.sync.dma_start(out=th0t[:], in_=th0_d[:])
                nc.sync.dma_start(out=al0t[:], in_=al0_d[:])
                nc.sync.dma_start(out=dt0t[:], in_=dt0_d[:])
                nc.sync.dma_start(out=th1t[:], in_=th1_d[:])
                nc.sync.dma_start(out=al1t[:], in_=al1_d[:])
                nc.sync.dma_start(out=dt1t[:], in_=dt1_d[:])

                emit_trig(th0t, al0t, tg0, wsc0, aw0, F0, T0)
                SC.activation(out=d16_0[:], in_=dt0t[:], func=CPY)
                emit_fold(X0, tg0, F0, T0)
                V.tensor_copy(out=apx(X0, 0, (F0, 9), (J0, CHI)),
                              in_=apx(rj, 0, (1, 9), (9, CHI)))
                emit_trig(th1t, al1t, tg1, wsc1, aw1, F1, T1)
                SC.activation(out=d16_1[:], in_=dt1t[:], func=CPY)

            # ======== scans + rest ========
            with tc.tile_pool(name="px1", bufs=1) as px:
                X1 = px.tile([P, T1 * F1 * 9], f16)
                w1 = px.tile([P, T1 * F1 * 3], f16)
                tW1_1 = px.tile([P, T1 * F1], f16)
                tW2_1 = px.tile([P, T1 * F1], f16)
                tA1 = px.tile([P, 4 * F1], f16)
                tB1 = px.tile([P, 6 * F1], f16)
                tC1 = px.tile([P, 6 * F1], f16)
                bht1 = px.tile([P, F1 * 12], f16)
                rx1 = px.tile([P, F1 * 12], f16)
                rx16_1 = px.tile([P, F1 * 12], f16)
                tA1h = px.tile([P, 96 * CHI * S1], f16)
                tB1h = px.tile([P, 96 * CHI * S1], f16)

                V.memset(apx(X1, 2 * F1, (1, F1)), 0.0)
                emit_fold(X1, tg1, F1, T1)

                def scan_i(X, tA, tB, tC, F, t, i):
                    pb = (t - 1) * 9 * F
                    cb = t * 9 * F
                    if i == 0:
                        V.tensor_mul(
                            out=apx(tA, 0, (2 * F, 2), (F, 2), (1, F)),
                            in0=apx(X, pb, (3 * F, 2), (0, 2), (1, F)),
                            in1=apx(X, cb, (0, 2), (F, 2), (1, F)))
                    elif i == 1:
                        V.tensor_mul(
                            out=apx(tB, 0, (3 * F, 2), (F, 3), (1, F)),
                            in0=apx(X, pb + F, (3 * F, 2), (0, 3), (1, F)),
                            in1=apx(X, cb + 3 * F, (0, 2), (F, 3), (1, F)))
                    elif i == 2:
                        V.tensor_mul(
                            out=apx(tC, 0, (3 * F, 2), (F, 3), (1, F)),
                            in0=apx(X, pb + 2 * F, (3 * F, 2), (0, 3),
                                    (1, F)),
                            in1=apx(X, cb + 6 * F, (0, 2), (F, 3), (1, F)))
                    elif i == 3:
                        V.tensor_add(
                            out=apx(tA, 0, (2 * F, 2), (F, 2), (1, F)),
                            in0=apx(tA, 0, (2 * F, 2), (F, 2), (1, F)),
                            in1=apx(tB, 0, (3 * F, 2), (F, 2), (1, F)))
                    elif i == 4:
                        V.tensor_add(
                            out=apx(X, cb, (3 * F, 2), (F, 2), (1, F)),
                            in0=apx(tA, 0, (2 * F, 2), (F, 2), (1, F)),
                            in1=apx(tC, 0, (3 * F, 2), (F, 2), (1, F)))
                    else:
                        V.tensor_add(
                            out=apx(X, cb + 2 * F, (3 * F, 2), (1, F)),
                            in0=apx(tB, 2 * F, (3 * F, 2), (1, F)),
                            in1=apx(tC, 2 * F, (3 * F, 2), (1, F)))

                for t in range(1, T0):
                    for i in range(6):
                        scan_i(X0, tA0, tB0, tC0, F0, t, i)
                        if t < T1:
                            scan_i(X1, tA1, tB1, tC1, F1, t, i)

                def emit_w(X, w, tW1, tW2, d16, F, T):
                    V.tensor_mul(out=apx(tW1, 0, (F, T), (1, F)),
                                 in0=apx(X, F, (9 * F, T), (1, F)),
                                 in1=apx(X, 5 * F, (9 * F, T), (1, F)))
                    V.tensor_mul(out=apx(tW2, 0, (F, T), (1, F)),
                                 in0=apx(X, 2 * F, (9 * F, T), (1, F)),
                                 in1=apx(X, 4 * F, (9 * F, T), (1, F)))
                    V.tensor_sub(out=apx(tW1, 0, (F, T), (1, F)),
                                 in0=apx(tW1, 0, (F, T), (1, F)),
                                 in1=apx(tW2, 0, (F, T), (1, F)))
                    V.tensor_mul(out=apx(w, 2 * F, (3 * F, T), (1, F)),
                                 in0=apx(tW1, 0, (F, T), (1, F)),
                                 in1=apx(d16, 0, (F, T), (1, F)))
                    V.tensor_mul(out=apx(w, 0, (3 * F, T), (F, 2), (1, F)),
                                 in0=apx(X, 0, (9 * F, T), (3 * F, 2),
                                         (1, F)),
                                 in1=apx(d16, 0, (F, T), (0, 2), (1, F)))

                emit_w(X0, w0, tW1_0, tW2_0, d16_0, F0, T0)
                # jump translation into w0 slab0 lanes chi*J0
                V.tensor_copy(out=apx(w0, 0, (F0, 3), (J0, CHI)),
                              in_=apx(jd, 0, (1, 3), (9, CHI)))

                # cumsum0 is a serial add chain: interleave gen1's w phase
                # and cumsum into its dependency gaps
                fillers = []
                fillers.append(lambda: emit_w_i1(0))
                fillers.append(lambda: emit_w_i1(1))
                fillers.append(lambda: emit_w_i1(2))
                fillers.append(lambda: emit_w_i1(3))
                fillers.append(lambda: emit_w_i1(4))
                for tq in range(1, T1):
                    fillers.append(
                        lambda tq=tq: V.tensor_add(
                            out=apx(w1, tq * 3 * F1, (1, 3 * F1)),
                            in0=apx(w1, tq * 3 * F1, (1, 3 * F1)),
                            in1=apx(w1, (tq - 1) * 3 * F1, (1, 3 * F1))))

                def emit_w_i1(i):
                    X, w, tW1, tW2, d16, F, T = (X1, w1, tW1_1, tW2_1,
                                                 d16_1, F1, T1)
                    if i == 0:
                        V.tensor_mul(out=apx(tW1, 0, (F, T), (1, F)),
                                     in0=apx(X, F, (9 * F, T), (1, F)),
                                     in1=apx(X, 5 * F, (9 * F, T), (1, F)))
                    elif i == 1:
                        V.tensor_mul(out=apx(tW2, 0, (F, T), (1, F)),
                                     in0=apx(X, 2 * F, (9 * F, T), (1, F)),
                                     in1=apx(X, 4 * F, (9 * F, T), (1, F)))
                    elif i == 2:
                        V.tensor_sub(out=apx(tW1, 0, (F, T), (1, F)),
                                     in0=apx(tW1, 0, (F, T), (1, F)),
                                     in1=apx(tW2, 0, (F, T), (1, F)))
                    elif i == 3:
                        V.tensor_mul(out=apx(w, 2 * F, (3 * F, T), (1, F)),
                                     in0=apx(tW1, 0, (F, T), (1, F)),
                                     in1=apx(d16, 0, (F, T), (1, F)))
                    else:
                        V.tensor_mul(out=apx(w, 0, (3 * F, T), (F, 2),
                                             (1, F)),
                                     in0=apx(X, 0, (9 * F, T), (3 * F, 2),
                                             (1, F)),
                                     in1=apx(d16, 0, (F, T), (0, 2), (1, F)))

                fi = 0
                for t in range(1, T0):
                    V.tensor_add(
                        out=apx(w0, t * 3 * F0, (1, 3 * F0)),
                        in0=apx(w0, t * 3 * F0, (1, 3 * F0)),
                        in1=apx(w0, (t - 1) * 3 * F0, (1, 3 * F0)))
                    if fi < len(fillers):
                        fillers[fi]()
                        fi += 1
                while fi < len(fillers):
                    fillers[fi]()
                    fi += 1

                # a32: in-block HT of branch root (lane j=32 per chi, t=0)
                V.tensor_copy(out=apx(a32t, 0, (12, CHI), (4, 2), (1, 3)),
                              in_=apx(X0, 32, (J0, CHI), (3 * F0, 2),
                                      (F0, 3)))
                SC.copy(out=apx(a32t, 8, (12, CHI)),
                        in_=apx(tW1_0, 32, (J0, CHI)))
                for dsti, (e1, e2), (e3, e4) in ((9, (2, 3), (0, 5)),
                                                 (10, (0, 4), (1, 3))):
                    V.tensor_mul(out=apx(tAh, 0, (1, CHI)),
                                 in0=apx(X0, 32 + e1 * F0, (J0, CHI)),
                                 in1=apx(X0, 32 + e2 * F0, (J0, CHI)))
                    V.tensor_mul(out=apx(tBh, 0, (1, CHI)),
                                 in0=apx(X0, 32 + e3 * F0, (J0, CHI)),
                                 in1=apx(X0, 32 + e4 * F0, (J0, CHI)))
                    V.tensor_sub(out=apx(a32t, dsti, (12, CHI)),
                                 in0=apx(tAh, 0, (1, CHI)),
                                 in1=apx(tBh, 0, (1, CHI)))
                V.tensor_copy(out=apx(a32t, 3, (12, CHI), (4, 3)),
                              in_=apx(w0, 32, (J0, CHI), (F0, 3)))

                # block-total HTs -> fp32 packed bht
                def emit_bht(X, w, tW1, bht, F, T):
                    base = (T - 1) * 9 * F
                    SC.copy(out=apx(bht, 0, (12, F), (4, 2), (1, 3)),
                            in_=apx(X, base, (1, F), (3 * F, 2), (F, 3)))
                    SC.copy(out=apx(bht, 8, (12, F)),
                            in_=apx(tW1, (T - 1) * F, (1, F)))
                    for dsti, (e1, e2), (e3, e4) in ((9, (2, 3), (0, 5)),
                                                     (10, (0, 4), (1, 3))):
                        V.tensor_mul(out=apx(tAh, 0, (1, F)),
                                     in0=apx(X, base + e1 * F, (1, F)),
                                     in1=apx(X, base + e2 * F, (1, F)))
                        V.tensor_mul(out=apx(tBh, 0, (1, F)),
                                     in0=apx(X, base + e3 * F, (1, F)),
                                     in1=apx(X, base + e4 * F, (1, F)))
                        V.tensor_sub(out=apx(bht, dsti, (12, F)),
                                     in0=apx(tAh, 0, (1, F)),
                                     in1=apx(tBh, 0, (1, F)))
                    SC.copy(out=apx(bht, 3, (12, F), (4, 3)),
                            in_=apx(w, (T - 1) * 3 * F, (1, F), (F, 3)))

                emit_bht(X0, w0, tW1_0, bht0, F0, T0)
                emit_bht(X1, w1, tW1_1, bht1, F1, T1)

                # ---- hierarchy (fp32, as v1) ----
                LPS0 = (U0 + 1) * 12
                LPS1 = (U1 + 1) * 12
                V.tensor_copy(out=apx(lp2_0, 12, (LPS0, CHI * S0), (1, 12)),
                              in_=apx(bht0, 0, (U0 * 12, CHI * S0), (1, 12)))
                SC.copy(out=apx(lp2_1, 12, (LPS1, CHI * S1), (1, 12)),
                        in_=apx(bht1, 0, (U1 * 12, CHI * S1), (1, 12)))
                for u in range(1, U0):
                    compose_1d(V, CHI * S0,
                               a_off=u * 12, a_step=LPS0,
                               b_off=u * 12, b_step=U0 * 12,
                               o_off=(u + 1) * 12, o_step=LPS0,
                               tA=tAh, tB=tBh,
                               a_tile=lp2_0, b_tile=bht0, o_tile=lp2_0)
                    if u < U1:
                        compose_1d(V, CHI * S1,
                                   a_off=u * 12, a_step=LPS1,
                                   b_off=u * 12, b_step=U1 * 12,
                                   o_off=(u + 1) * 12, o_step=LPS1,
                                   tA=tA1h, tB=tB1h,
                                   a_tile=lp2_1, b_tile=bht1, o_tile=lp2_1)
                for sidx in range(1, S0):
                    compose_1d(V, CHI,
                               a_off=(sidx - 1) * 12, a_step=S0 * 12,
                               b_off=(sidx - 1) * LPS0 + U0 * 12,
                               b_step=S0 * LPS0,
                               o_off=sidx * 12, o_step=S0 * 12,
                               tA=tAh, tB=tBh,
                               a_tile=spx0, b_tile=lp2_0, o_tile=spx0)
                excl_blocks(V, SC, CHI * S0, U0, LPS0, spx0, lp2_0, rx0,
                            tAh, tBh)
                compose_1d(V, CHI,
                           a_off=32 * 12, a_step=J0 * 12,
                           b_off=0, b_step=12,
                           o_off=0, o_step=12,
                           tA=tAh, tB=tBh,
                           a_tile=rx0, b_tile=a32t, o_tile=rbr)
                SC.copy(out=apx(spx1, 0, (S1 * 12, CHI), (1, 12)),
                        in_=apx(rbr, 0, (12, CHI), (1, 12)))
                # rx -> planar fp16 for the down transform
                V.tensor_copy(out=apx(rx16_0, 0, (F0, 12), (1, F0)),
                              in_=apx(rx0, 0, (1, 12), (12, F0)))

                def down_i(w, rx16, X, tmpoff, F, T, i):
                    xyz = apx(X, 0, (3 * F, T), (F, 3), (1, F))
                    tmp = apx(X, tmpoff, (3 * F, T), (F, 3), (1, F))

                    def rxk(k):
                        return apx(rx16, k * F, (0, T), (4 * F, 3), (1, F))

                    def wk(k):
                        return apx(w, k * F, (3 * F, T), (0, 3), (1, F))

                    if i == 0:
                        V.tensor_mul(out=xyz, in0=rxk(0), in1=wk(0))
                    elif i == 1:
                        V.tensor_mul(out=tmp, in0=rxk(1), in1=wk(1))
                    elif i == 2:
                        V.tensor_add(out=xyz, in0=xyz, in1=tmp)
                    elif i == 3:
                        V.tensor_mul(out=tmp, in0=rxk(2), in1=wk(2))
                    elif i == 4:
                        V.tensor_add(out=xyz, in0=xyz, in1=tmp)
                    else:
                        V.tensor_add(out=xyz, in0=xyz, in1=rxk(3))

                # tail: interleave gen1 level-3/excl with the down-g0
                # instructions to hide the tiny serial composes
                down0_iter = iter(range(6))
                for sidx in range(1, S1):
                    compose_1d(V, CHI,
                               a_off=(sidx - 1) * 12, a_step=S1 * 12,
                               b_off=(sidx - 1) * LPS1 + U1 * 12,
                               b_step=S1 * LPS1,
                               o_off=sidx * 12, o_step=S1 * 12,
                               tA=tA1h, tB=tB1h,
                               a_tile=spx1, b_tile=lp2_1, o_tile=spx1)
                    i = next(down0_iter, None)
                    if i is not None:
                        down_i(w0, rx16_0, X0, 3 * F0 * T0, F0, T0, i)
                excl_blocks(V, SC, CHI * S1, U1, LPS1, spx1, lp2_1, rx1,
                            tA1h, tB1h)
                V.tensor_copy(out=apx(rx16_1, 0, (F1, 12), (1, F1)),
                              in_=apx(rx1, 0, (1, 12), (12, F1)))
                for i in down0_iter:
                    down_i(w0, rx16_0, X0, 3 * F0 * T0, F0, T0, i)
                nc.sync.dma_start(
                    out=AP(kin0_d, 0, [[F0 * T0 * 3, P], [1, F0 * T0 * 3]]),
                    in_=apx(X0, 0, (1, F0 * T0 * 3)))
                for i in range(6):
                    down_i(w1, rx16_1, X1, 3 * F1 * T1, F1, T1, i)
                nc.sync.dma_start(
                    out=AP(kin1_d, 0, [[F1 * T1 * 3, P], [1, F1 * T1 * 3]]),
                    in_=apx(X1, 0, (1, F1 * T1 * 3)))

    nc.compile()
    return nc


def get_program(repeat=1):
    key = ("nc", repeat)
    if key not in _CACHE:
        _CACHE[key] = _build_program(repeat)
    return _CACHE[key]


# ------------------------------------------------------------------- host
def _shard_inputs(dofs, doftype):
    """Per-core inputs, pre-transposed to device t-major lane order.

    Device order per partition p: index t*F + chi*J + j for atom
    (chi, j, t); host layout [P, CHI*L].  Alpha is pre-folded on the host:
    alpha_p = phi_c(parent) + phi_p(p) (chain starts: phi_p only; branch
    roots fold phi_c of gen0 atom 384)."""
    def to_dev(arr, J, T):
        # arr: [C_core, L] (chain-major) -> [P, T*CHI*J]
        a = arr.reshape(CHI, P, J, T)
        return np.ascontiguousarray(
            a.transpose(1, 3, 0, 2).reshape(P, CHI * J * T))

    chain_starts = 1 + np.arange(C0, dtype=np.int64) * L0
    jd_all = np.ascontiguousarray(dofs[chain_starts])       # [C0, 9]

    ph0 = dofs[1:BOFF, 0].reshape(C0, L0)
    th0 = dofs[1:BOFF, 1].reshape(C0, L0)
    d0 = dofs[1:BOFF, 2].reshape(C0, L0)
    pc0 = dofs[1:BOFF, 3].reshape(C0, L0)
    al0 = np.empty_like(ph0)
    al0[:, 0] = 0.0
    al0[:, 1] = ph0[:, 1]
    al0[:, 2:] = ph0[:, 2:] + pc0[:, 1:-1]

    ph1 = dofs[BOFF:, 0].reshape(C1, L1)
    th1 = dofs[BOFF:, 1].reshape(C1, L1)
    d1 = dofs[BOFF:, 2].reshape(C1, L1)
    pc1 = dofs[BOFF:, 3].reshape(C1, L1)
    al1 = np.empty_like(ph1)
    al1[:, 0] = ph1[:, 0] + pc0[:, 384]
    al1[:, 1:] = ph1[:, 1:] + pc1[:, :-1]

    in_maps = []
    for core in range(NCORES):
        s0 = slice(core * CH0, (core + 1) * CH0)
        s1 = slice(core * CH1, (core + 1) * CH1)
        jd = np.ascontiguousarray(
            jd_all[s0].reshape(CHI, P, 9).transpose(1, 0, 2)
            .reshape(P, CHI * 9))
        in_maps.append({
            "th0": to_dev(th0[s0], J0, T0),
            "al0": to_dev(al0[s0], J0, T0),
            "dt0": to_dev(d0[s0], J0, T0),
            "th1": to_dev(th1[s1], J1, T1),
            "al1": to_dev(al1[s1], J1, T1),
            "dt1": to_dev(d1[s1], J1, T1),
            "jd": jd,
        })
    return in_maps


def _lane_ids(id_idx, core):
    """id_idx values of this core's atoms in device lane order (p, f, t)."""
    ids0 = (id_idx[core * A0:(core + 1) * A0]
            .reshape(CHI, P, L0).transpose(1, 0, 2).ravel())
    ids1 = (id_idx[BOFF - 1 + core * A1: BOFF - 1 + (core + 1) * A1]
            .reshape(CHI, P, L1).transpose(1, 0, 2).ravel())
    return ids0, ids1


def _structure_ok(doftype, gen0_paths, gen1_paths):
    chain_starts = 1 + np.arange(C0, dtype=np.int64) * L0
    g0 = np.concatenate(
        [np.zeros((C0, 1), np.int64), chain_starts[:, None] + np.arange(L0)],
        axis=1)
    if not np.array_equal(gen0_paths, g0.astype(gen0_paths.dtype)):
        return False
    branch_roots = chain_starts + L0 // 2
    g1 = np.concatenate(
        [branch_roots[:, None],
         BOFF + (np.arange(C1, dtype=np.int64) * L1)[:, None] + np.arange(L1)],
        axis=1)
    if not np.array_equal(gen1_paths, g1.astype(gen1_paths.dtype)):
        return False
    if doftype[0] != 0:
        return False
    if not np.all(doftype[chain_starts] == 1):
        return False
    dt = doftype.copy()
    dt[chain_starts] = 2
    if not np.all(dt[1:] == 2):
        return False
    return True


def _numpy_fallback(dofs, doftype, gen0_paths, gen1_paths, id_idx):
    """Exact numpy port of the reference (slow path, safety net)."""
    def rx(a):
        c, s = np.cos(a), np.sin(a)
        o, z = np.ones_like(a), np.zeros_like(a)
        return np.stack([np.stack([o, z, z, z], -1), np.stack([z, c, -s, z], -1),
                         np.stack([z, s, c, z], -1), np.stack([z, z, z, o], -1)], -2)

    def ry(a):
        c, s = np.cos(a), np.sin(a)
        o, z = np.ones_like(a), np.zeros_like(a)
        return np.stack([np.stack([c, z, s, z], -1), np.stack([z, o, z, z], -1),
                         np.stack([-s, z, c, z], -1), np.stack([z, z, z, o], -1)], -2)

    def rz(a):
        c, s = np.cos(a), np.sin(a)
        o, z = np.ones_like(a), np.zeros_like(a)
        return np.stack([np.stack([c, -s, z, z], -1), np.stack([s, c, z, z], -1),
                         np.stack([z, z, o, z], -1), np.stack([z, z, z, o], -1)], -2)

    def trans(x, y, z):
        o, zr = np.ones_like(x), np.zeros_like(x)
        return np.stack([np.stack([o, zr, zr, x], -1), np.stack([zr, o, zr, y], -1),
                         np.stack([zr, zr, o, z], -1), np.stack([zr, zr, zr, o], -1)], -2)

    dofs = dofs.astype(np.float32)
    phi_p, theta, d, phi_c = dofs[:, 0], dofs[:, 1], dofs[:, 2], dofs[:, 3]
    z = np.zeros_like(d)
    bond = rx(phi_p) @ rz(np.pi - theta) @ trans(d, z, z) @ rx(phi_c)
    rot = lambda a, b, c: rz(c) @ ry(b) @ rx(a)
    jump = (trans(dofs[:, 0], dofs[:, 1], dofs[:, 2])
            @ rot(dofs[:, 3], dofs[:, 4], dofs[:, 5])
            @ rot(dofs[:, 6], dofs[:, 7], dofs[:, 8]))
    eye = np.broadcast_to(np.eye(4, dtype=dofs.dtype), bond.shape)
    dt = doftype[:, None, None]
    hts = np.where(dt == 1, jump, np.where(dt == 2, bond, eye)).astype(np.float32)
    for paths in (gen0_paths, gen1_paths):
        seg = hts[paths]
        out = np.empty_like(seg)
        out[:, 0] = seg[:, 0]
        for i in range(1, seg.shape[1]):
            out[:, i] = out[:, i - 1] @ seg[:, i]
        hts[paths] = out
    kincoords = hts[:, :3, 3]
    coords = np.zeros((N - 1, 3), dtype=dofs.dtype)
    coords[np.asarray(id_idx)] = kincoords[1:]
    return coords


def kernel(dofs, doftype, gen0_paths, gen1_paths, id_idx):
    dofs = np.asarray(dofs, dtype=np.float32)
    doftype = np.asarray(doftype, dtype=np.int32)
    gen0_paths = np.asarray(gen0_paths)
    gen1_paths = np.asarray(gen1_paths)
    id_idx = np.asarray(id_idx, dtype=np.int32)

    if not _structure_ok(doftype, gen0_paths, gen1_paths):
        return _numpy_fallback(dofs, doftype, gen0_paths, gen1_paths, id_idx)

    from concourse.bass_utils import run_bass_kernel_spmd

    nc = get_program()
    in_maps = _shard_inputs(dofs, doftype)
    res = run_bass_kernel_spmd(nc, in_maps, core_ids=list(range(NCORES)))
    out = np.empty((N - 1, 3), dtype=np.float32)
    for core in range(NCORES):
        ids0, ids1 = _lane_ids(id_idx, core)
        k0 = res.results[core]["kin0"].astype(np.float32)
        k0 = k0.reshape(P, T0, 3, F0).transpose(0, 3, 1, 2).reshape(-1, 3)
        k1 = res.results[core]["kin1"].astype(np.float32)
        k1 = k1.reshape(P, T1, 3, F1).transpose(0, 3, 1, 2).reshape(-1, 3)
        out[ids0] = k0
        out[ids1] = k1
    return out


# revision 14
# speedup vs baseline: 1.1061x; 1.1061x over previous
"""Trainium2 Bass kernel for nn_KinematicOperation — v3: fp16 planar DVE.

Same blocked-scan algorithm as v1, but the lane-parallel phases (bond fold,
level-1 scan, w, cumsum, down-transform) run in fp16 with entry-PLANAR
layouts (inner dim = lanes, unit stride, 4B aligned), which engages the
DVE's 2x_1P mode: tensor_tensor at 2 elem/cycle.  Angles stay fp32 through
the ACT sine (fp16 angle rounding would dominate the error budget); sin/cos
outputs and all downstream per-atom products are fp16.  The block-level
hierarchy (bht, levels, excl) stays fp32 packed as in v1.  ACT does the
transposing casts (trig planes, d-column, rx expansion, output repack).

Layouts per partition (generation with F lanes, T slabs):
  X[t][e][f]   e in 0..8: 3x3 entries row-major; slabs t (fp16)
  w[t][c][f]   c in 0..2 (fp16)
  trig planes nm[t*F + f] (fp16), angles atom-major fp32
  bht/lp2/spx/rx packed 12-elem HTs per lane (fp32), rx16 planar fp16
"""

import os
import sys

import numpy as np

for _p in ("/opt/trn_rl_repo", "/root/.axon_site/_ro/trn_rl_repo"):
    if os.path.isdir(_p) and _p not in sys.path:
        sys.path.insert(0, _p)

C0, L0 = 2048, 768
C1, L1 = 2048, 256
N = 1 + C0 * L0 + C1 * L1
BOFF = 1 + C0 * L0
NCORES = 8
P = 128
CHI = 2
CH0 = C0 // NCORES
CH1 = C1 // NCORES
A0 = CH0 * L0
A1 = CH1 * L1

T0, J0, S0, U0 = 12, 64, 8, 8
F0 = CHI * J0
T1, J1, S1, U1 = 8, 32, 4, 8
F1 = CHI * J1

PI = float(np.pi)

_CACHE = {}


def _build_program(repeat=1):
    from concourse import bacc, mybir, tile
    from concourse.bass import AP

    f32 = mybir.dt.float32
    f16 = mybir.dt.float16
    SIN = mybir.ActivationFunctionType.Sin
    ABS = mybir.ActivationFunctionType.Abs
    CPY = mybir.ActivationFunctionType.Copy

    nc = bacc.Bacc("TRN2", target_bir_lowering=False, debug=False)

    th0_d = nc.dram_tensor("th0", [P, CHI * L0], f32, kind="ExternalInput")
    al0_d = nc.dram_tensor("al0", [P, CHI * L0], f32, kind="ExternalInput")
    dt0_d = nc.dram_tensor("dt0", [P, CHI * L0], f32, kind="ExternalInput")
    th1_d = nc.dram_tensor("th1", [P, CHI * L1], f32, kind="ExternalInput")
    al1_d = nc.dram_tensor("al1", [P, CHI * L1], f32, kind="ExternalInput")
    dt1_d = nc.dram_tensor("dt1", [P, CHI * L1], f32, kind="ExternalInput")
    jd_d = nc.dram_tensor("jd", [P, CHI * 9], f32, kind="ExternalInput")
    kin0_d = nc.dram_tensor("kin0", [P, F0 * T0 * 3], f16,
                            kind="ExternalOutput")
    kin1_d = nc.dram_tensor("kin1", [P, F1 * T1 * 3], f16,
                            kind="ExternalOutput")

    def apx(tl, off, *dims):
        t = tl[:] if not isinstance(tl, AP) else tl
        return AP(t.tensor, t.offset + off,
                  [[t.ap[0][0], P]] + [list(d) for d in dims])

    def compose_1d(E, lanes, a_off, a_step, b_off, b_step, o_off, o_step,
                   tA, tB, a_tile, b_tile, o_tile):
        for k, dst in ((0, tA), (1, tB)):
            E.tensor_mul(
                out=apx(dst, 0, (12, lanes), (4, 3), (1, 4)),
                in0=apx(a_tile, a_off + k, (a_step, lanes), (4, 3), (0, 4)),
                in1=apx(b_tile, b_off + 4 * k, (b_step, lanes), (0, 3), (1, 4)),
            )
        E.tensor_add(
            out=apx(tA, 0, (12, lanes), (1, 12)),
            in0=apx(tA, 0, (12, lanes), (1, 12)),
            in1=apx(tB, 0, (12, lanes), (1, 12)))
        E.tensor_mul(
            out=apx(tB, 0, (12, lanes), (4, 3), (1, 4)),
            in0=apx(a_tile, a_off + 2, (a_step, lanes), (4, 3), (0, 4)),
            in1=apx(b_tile, b_off + 8, (b_step, lanes), (0, 3), (1, 4)),
        )
        E.tensor_add(
            out=apx(o_tile, o_off, (o_step, lanes), (1, 12)),
            in0=apx(tA, 0, (12, lanes), (1, 12)),
            in1=apx(tB, 0, (12, lanes), (1, 12)),
        )
        E.tensor_add(
            out=apx(o_tile, o_off + 3, (o_step, lanes), (4, 3)),
            in0=apx(o_tile, o_off + 3, (o_step, lanes), (4, 3)),
            in1=apx(a_tile, a_off + 3, (a_step, lanes), (4, 3)),
        )

    def excl_blocks(E, SC, CS, U, LPS, spx, lp2, rx, tA, tB):
        SC.copy(out=apx(rx, 0, (U * 12, CS), (1, 12)),
                in_=apx(spx, 0, (12, CS), (1, 12)))
        UM = U - 1
        for i in range(3):
            for k, dst in ((0, tA), (1, tB)):
                E.tensor_mul(
                    out=apx(dst, 4 * i, (96, CS), (12, UM), (1, 4)),
                    in0=apx(spx, 4 * i + k, (12, CS), (0, UM), (0, 4)),
                    in1=apx(lp2, 12 + 4 * k, (LPS, CS), (12, UM), (1, 4)))
            E.tensor_add(
                out=apx(tA, 4 * i, (96, CS), (12, UM), (1, 4)),
                in0=apx(tA, 4 * i, (96, CS), (12, UM), (1, 4)),
                in1=apx(tB, 4 * i, (96, CS), (12, UM), (1, 4)))
            E.tensor_mul(
                out=apx(tB, 4 * i, (96, CS), (12, UM), (1, 4)),
                in0=apx(spx, 4 * i + 2, (12, CS), (0, UM), (0, 4)),
                in1=apx(lp2, 12 + 8, (LPS, CS), (12, UM), (1, 4)))
            E.tensor_add(
                out=apx(rx, 12 + 4 * i, (96, CS), (12, UM), (1, 4)),
                in0=apx(tA, 4 * i, (96, CS), (12, UM), (1, 4)),
                in1=apx(tB, 4 * i, (96, CS), (12, UM), (1, 4)))
        E.tensor_add(
            out=apx(rx, 12 + 3, (96, CS), (12, UM), (4, 3)),
            in0=apx(rx, 12 + 3, (96, CS), (12, UM), (4, 3)),
            in1=apx(spx, 3, (12, CS), (0, UM), (4, 3)))

    with tile.TileContext(nc) as tc:
      for _rep in range(repeat):
        with tc.tile_pool(name="main", bufs=1) as mp:
            X0 = mp.tile([P, T0 * F0 * 9], f16)
            w0 = mp.tile([P, T0 * F0 * 3], f16)
            tW1_0 = mp.tile([P, T0 * F0], f16)
            tW2_0 = mp.tile([P, T0 * F0], f16)
            tA0 = mp.tile([P, 4 * F0], f16)
            tB0 = mp.tile([P, 6 * F0], f16)
            tC0 = mp.tile([P, 6 * F0], f16)
            tg0 = {nm: mp.tile([P, T0 * F0], f16, name=f"tg0_{nm}")
                   for nm in ("sa", "ca", "st", "ct")}
            apl0 = mp.tile([P, CHI * L0], f32)
            wsc0b = mp.tile([P, CHI * L0], f32)
            wsc0 = mp.tile([P, CHI * L0], f32)
            aw0 = mp.tile([P, CHI * L0], f32)
            d16_0 = mp.tile([P, T0 * F0], f16)
            tg1 = {nm: mp.tile([P, T1 * F1], f16, name=f"tg1_{nm}")
                   for nm in ("sa", "ca", "st", "ct")}
            apl1 = mp.tile([P, CHI * L1], f32)
            wsc1b = mp.tile([P, CHI * L1], f32)
            wsc1 = mp.tile([P, CHI * L1], f32)
            aw1 = mp.tile([P, CHI * L1], f32)
            d16_1 = mp.tile([P, T1 * F1], f16)
            bht0 = mp.tile([P, F0 * 12], f16)
            rx0 = mp.tile([P, F0 * 12], f16)
            rx16_0 = mp.tile([P, F0 * 12], f16)
            lp2_0 = mp.tile([P, CHI * S0 * (U0 + 1) * 12], f16)
            spx0 = mp.tile([P, CHI * S0 * 12], f16)
            lp2_1 = mp.tile([P, CHI * S1 * (U1 + 1) * 12], f16)
            spx1 = mp.tile([P, CHI * S1 * 12], f16)
            tAh = mp.tile([P, 96 * CHI * S0], f16)
            tBh = mp.tile([P, 96 * CHI * S0], f16)
            a32t = mp.tile([P, CHI * 12], f16)
            rbr = mp.tile([P, CHI * 12], f16)
            jd = mp.tile([P, CHI * 9], f32)
            jang = mp.tile([P, CHI * 2 * 3], f32)
            jsin = mp.tile([P, CHI * 2 * 3], f32)
            jcos = mp.tile([P, CHI * 2 * 3], f32)
            re_ = mp.tile([P, CHI * 2 * 9], f32)
            rj = mp.tile([P, CHI * 9], f32)
            jtmp = mp.tile([P, CHI * 2 * 9], f32)
            halfpi = mp.tile([P, 1], f32)

            V = nc.vector
            SC = nc.scalar

            nc.sync.dma_start(out=jd[:], in_=jd_d[:])
            V.memset(halfpi[:], PI / 2)

            # prefill identities (fp32 hierarchy tiles)
            V.memset(lp2_0[:], 0.0)
            V.memset(apx(lp2_0, 0, ((U0 + 1) * 12, CHI * S0), (5, 3)), 1.0)
            V.memset(spx0[:], 0.0)
            V.memset(apx(spx0, 0, (S0 * 12, CHI), (5, 3)), 1.0)
            V.memset(lp2_1[:], 0.0)
            V.memset(apx(lp2_1, 0, ((U1 + 1) * 12, CHI * S1), (5, 3)), 1.0)
            V.memset(apx(X0, 2 * F0, (1, F0)), 0.0)  # slab0 e=2 plane

            # ---- JUMP HT build (fp32, tiny) ----
            V.tensor_copy(out=jang[:], in_=apx(jd, 3, (9, CHI), (3, 2),
                                               (1, 3)))
            V.add_range_wrap(out=jsin[:], in_=jang[:], shift=0.0,
                             bound=PI, period=2 * PI)
            SC.activation(out=jsin[:], in_=jsin[:], func=SIN)
            V.add_range_wrap(out=jcos[:], in_=jang[:], shift=PI / 2,
                             bound=PI, period=2 * PI)
            SC.activation(out=jcos[:], in_=jcos[:], func=SIN)

            CR = CHI * 2

            def sc_(tl, ang):
                return apx(tl, ang, (3, CR))

            def re(e):
                return apx(re_, e, (9, CR))

            def jt1(e):
                return apx(jtmp, e, (9, CR))

            sa_ = lambda: sc_(jsin, 0)
            sb = lambda: sc_(jsin, 1)
            s_c = lambda: sc_(jsin, 2)
            ca_ = lambda: sc_(jcos, 0)
            cb = lambda: sc_(jcos, 1)
            c_c = lambda: sc_(jcos, 2)
            V.tensor_mul(out=re(0), in0=c_c(), in1=cb())
            V.tensor_mul(out=jt1(0), in0=sb(), in1=sa_())
            V.tensor_mul(out=jt1(1), in0=sb(), in1=ca_())
            V.tensor_mul(out=jt1(2), in0=c_c(), in1=jt1(0))
            V.tensor_mul(out=jt1(3), in0=s_c(), in1=ca_())
            V.tensor_sub(out=re(1), in0=jt1(2), in1=jt1(3))
            V.tensor_mul(out=jt1(2), in0=c_c(), in1=jt1(1))
            V.tensor_mul(out=jt1(3), in0=s_c(), in1=sa_())
            V.tensor_add(out=re(2), in0=jt1(2), in1=jt1(3))
            V.tensor_mul(out=re(3), in0=s_c(), in1=cb())
            V.tensor_mul(out=jt1(2), in0=s_c(), in1=jt1(0))
            V.tensor_mul(out=jt1(3), in0=c_c(), in1=ca_())
            V.tensor_add(out=re(4), in0=jt1(2), in1=jt1(3))
            V.tensor_mul(out=jt1(2), in0=s_c(), in1=jt1(1))
            V.tensor_mul(out=jt1(3), in0=c_c(), in1=sa_())
            V.tensor_sub(out=re(5), in0=jt1(2), in1=jt1(3))
            V.tensor_scalar_mul(out=re(6), in0=sb(), scalar1=-1.0)
            V.tensor_mul(out=re(7), in0=cb(), in1=sa_())
            V.tensor_mul(out=re(8), in0=cb(), in1=ca_())
            V.tensor_mul(
                out=apx(rj, 0, (9, CHI), (3, 3), (1, 3)),
                in0=apx(re_, 0, (18, CHI), (3, 3), (0, 3)),
                in1=apx(re_, 9, (18, CHI), (0, 3), (1, 3)))
            V.tensor_mul(
                out=apx(jtmp, 0, (9, CHI), (3, 3), (1, 3)),
                in0=apx(re_, 1, (18, CHI), (3, 3), (0, 3)),
                in1=apx(re_, 12, (18, CHI), (0, 3), (1, 3)))
            V.tensor_add(out=rj[:, : CHI * 9], in0=rj[:, : CHI * 9],
                         in1=jtmp[:, : CHI * 9])
            V.tensor_mul(
                out=apx(jtmp, 0, (9, CHI), (3, 3), (1, 3)),
                in0=apx(re_, 2, (18, CHI), (3, 3), (0, 3)),
                in1=apx(re_, 15, (18, CHI), (0, 3), (1, 3)))
            V.tensor_add(out=rj[:, : CHI * 9], in0=rj[:, : CHI * 9],
                         in1=jtmp[:, : CHI * 9])

            # ======== trig (both gens): t-major fp32 inputs ========
            # host pre-transposes theta/alpha/d to device order (t-major)
            # and pre-folds alpha = phi_c(parent) + phi_p (incl. the branch
            # root fold), so trig is wrap + contiguous ACT sines only.
            def emit_trig(tht, alt, tg, w1s, w2s, w3s, w4s):
                # cos via a second shifted wrap on the (front-idle) DVE so
                # ACT runs only 4 contiguous sines per generation; four
                # scratch tiles avoid WAR stalls between V wraps and ACT
                V.add_range_wrap(out=w1s[:], in_=tht[:], shift=0.0,
                                 bound=PI, period=2 * PI)
                V.add_range_wrap(out=w2s[:], in_=tht[:], shift=PI / 2,
                                 bound=PI, period=2 * PI)
                V.add_range_wrap(out=w3s[:], in_=alt[:], shift=0.0,
                                 bound=PI, period=2 * PI)
                V.add_range_wrap(out=w4s[:], in_=alt[:], shift=PI / 2,
                                 bound=PI, period=2 * PI)
                SC.activation(out=tg["st"][:], in_=w1s[:], func=SIN)
                SC.activation(out=tg["ct"][:], in_=w2s[:], func=SIN)
                SC.activation(out=tg["sa"][:], in_=w3s[:], func=SIN)
                SC.activation(out=tg["ca"][:], in_=w4s[:], func=SIN)

            def emit_fold(X, tg, F, T):
                def tp(nm):
                    return apx(tg[nm], 0, (F, T), (1, F))

                def xo(e):
                    return apx(X, e * F, (9 * F, T), (1, F))

                V.tensor_scalar_mul(out=xo(0), in0=tp("ct"), scalar1=-1.0)
                V.tensor_scalar_mul(out=xo(1), in0=tp("st"), scalar1=-1.0)
                V.tensor_scalar_mul(out=xo(5), in0=tp("sa"), scalar1=-1.0)
                V.tensor_copy(out=xo(8), in_=tp("ca"))
                V.tensor_mul(out=xo(3), in0=tp("ca"), in1=tp("st"))
                V.tensor_mul(out=xo(4), in0=tp("ca"), in1=xo(0))
                V.tensor_mul(out=xo(6), in0=tp("sa"), in1=tp("st"))
                V.tensor_mul(out=xo(7), in0=tp("sa"), in1=xo(0))

            with tc.tile_pool(name="pdof", bufs=1) as pd:
                th0t = pd.tile([P, CHI * L0], f32)
                al0t = pd.tile([P, CHI * L0], f32)
                dt0t = pd.tile([P, CHI * L0], f32)
                th1t = pd.tile([P, CHI * L1], f32)
                al1t = pd.tile([P, CHI * L1], f32)
                dt1t = pd.tile([P, CHI * L1], f32)

                nc.sync.dma_start(out=th0t[:], in_=th0_d[:])
                nc.sync.dma_start(out=al0t[:], in_=al0_d[:])
                nc.sync.dma_start(out=dt0t[:], in_=dt0_d[:])
                nc.sync.dma_start(out=th1t[:], in_=th1_d[:])
                nc.sync.dma_start(out=al1t[:], in_=al1_d[:])
                nc.sync.dma_start(out=dt1t[:], in_=dt1_d[:])

                emit_trig(th0t, al0t, tg0, wsc0, aw0, apl0, wsc0b)
                SC.activation(out=d16_0[:], in_=dt0t[:], func=CPY)
                emit_fold(X0, tg0, F0, T0)
                V.tensor_copy(out=apx(X0, 0, (F0, 9), (J0, CHI)),
                              in_=apx(rj, 0, (1, 9), (9, CHI)))
                emit_trig(th1t, al1t, tg1, wsc1, aw1, apl1, wsc1b)
                SC.activation(out=d16_1[:], in_=dt1t[:], func=CPY)

            # ======== scans + rest ========
            with tc.tile_pool(name="px1", bufs=1) as px:
                X1 = px.tile([P, T1 * F1 * 9], f16)
                w1 = px.tile([P, T1 * F1 * 3], f16)
                tW1_1 = px.tile([P, T1 * F1], f16)
                tW2_1 = px.tile([P, T1 * F1], f16)
                tA1 = px.tile([P, 4 * F1], f16)
                tB1 = px.tile([P, 6 * F1], f16)
                tC1 = px.tile([P, 6 * F1], f16)
                bht1 = px.tile([P, F1 * 12], f16)
                rx1 = px.tile([P, F1 * 12], f16)
                rx16_1 = px.tile([P, F1 * 12], f16)
                tA1h = px.tile([P, 96 * CHI * S1], f16)
                tB1h = px.tile([P, 96 * CHI * S1], f16)

                V.memset(apx(X1, 2 * F1, (1, F1)), 0.0)
                emit_fold(X1, tg1, F1, T1)

                def scan_i(X, tA, tB, tC, F, t, i):
                    pb = (t - 1) * 9 * F
                    cb = t * 9 * F
                    if i == 0:
                        V.tensor_mul(
                            out=apx(tA, 0, (2 * F, 2), (F, 2), (1, F)),
                            in0=apx(X, pb, (3 * F, 2), (0, 2), (1, F)),
                            in1=apx(X, cb, (0, 2), (F, 2), (1, F)))
                    elif i == 1:
                        V.tensor_mul(
                            out=apx(tB, 0, (3 * F, 2), (F, 3), (1, F)),
                            in0=apx(X, pb + F, (3 * F, 2), (0, 3), (1, F)),
                            in1=apx(X, cb + 3 * F, (0, 2), (F, 3), (1, F)))
                    elif i == 2:
                        V.tensor_mul(
                            out=apx(tC, 0, (3 * F, 2), (F, 3), (1, F)),
                            in0=apx(X, pb + 2 * F, (3 * F, 2), (0, 3),
                                    (1, F)),
                            in1=apx(X, cb + 6 * F, (0, 2), (F, 3), (1, F)))
                    elif i == 3:
                        V.tensor_add(
                            out=apx(tA, 0, (2 * F, 2), (F, 2), (1, F)),
                            in0=apx(tA, 0, (2 * F, 2), (F, 2), (1, F)),
                            in1=apx(tB, 0, (3 * F, 2), (F, 2), (1, F)))
                    elif i == 4:
                        V.tensor_add(
                            out=apx(X, cb, (3 * F, 2), (F, 2), (1, F)),
                            in0=apx(tA, 0, (2 * F, 2), (F, 2), (1, F)),
                            in1=apx(tC, 0, (3 * F, 2), (F, 2), (1, F)))
                    else:
                        V.tensor_add(
                            out=apx(X, cb + 2 * F, (3 * F, 2), (1, F)),
                            in0=apx(tB, 2 * F, (3 * F, 2), (1, F)),
                            in1=apx(tC, 2 * F, (3 * F, 2), (1, F)))

                for t in range(1, T0):
                    for i in range(6):
                        scan_i(X0, tA0, tB0, tC0, F0, t, i)
                        if t < T1:
                            scan_i(X1, tA1, tB1, tC1, F1, t, i)

                def emit_w(X, w, tW1, tW2, d16, F, T):
                    V.tensor_mul(out=apx(tW1, 0, (F, T), (1, F)),
                                 in0=apx(X, F, (9 * F, T), (1, F)),
                                 in1=apx(X, 5 * F, (9 * F, T), (1, F)))
                    V.tensor_mul(out=apx(tW2, 0, (F, T), (1, F)),
                                 in0=apx(X, 2 * F, (9 * F, T), (1, F)),
                                 in1=apx(X, 4 * F, (9 * F, T), (1, F)))
                    V.tensor_sub(out=apx(tW1, 0, (F, T), (1, F)),
                                 in0=apx(tW1, 0, (F, T), (1, F)),
                                 in1=apx(tW2, 0, (F, T), (1, F)))
                    V.tensor_mul(out=apx(w, 2 * F, (3 * F, T), (1, F)),
                                 in0=apx(tW1, 0, (F, T), (1, F)),
                                 in1=apx(d16, 0, (F, T), (1, F)))
                    V.tensor_mul(out=apx(w, 0, (3 * F, T), (F, 2), (1, F)),
                                 in0=apx(X, 0, (9 * F, T), (3 * F, 2),
                                         (1, F)),
                                 in1=apx(d16, 0, (F, T), (0, 2), (1, F)))

                emit_w(X0, w0, tW1_0, tW2_0, d16_0, F0, T0)
                emit_w(X1, w1, tW1_1, tW2_1, d16_1, F1, T1)

                # jump translation into w0 slab0 lanes chi*J0
                V.tensor_copy(out=apx(w0, 0, (F0, 3), (J0, CHI)),
                              in_=apx(jd, 0, (1, 3), (9, CHI)))

                # a32: in-block HT of branch root (lane j=32 per chi, t=0)
                V.tensor_copy(out=apx(a32t, 0, (12, CHI), (4, 2), (1, 3)),
                              in_=apx(X0, 32, (J0, CHI), (3 * F0, 2),
                                      (F0, 3)))
                SC.copy(out=apx(a32t, 8, (12, CHI)),
                        in_=apx(tW1_0, 32, (J0, CHI)))
                for dsti, (e1, e2), (e3, e4) in ((9, (2, 3), (0, 5)),
                                                 (10, (0, 4), (1, 3))):
                    V.tensor_mul(out=apx(tAh, 0, (1, CHI)),
                                 in0=apx(X0, 32 + e1 * F0, (J0, CHI)),
                                 in1=apx(X0, 32 + e2 * F0, (J0, CHI)))
                    V.tensor_mul(out=apx(tBh, 0, (1, CHI)),
                                 in0=apx(X0, 32 + e3 * F0, (J0, CHI)),
                                 in1=apx(X0, 32 + e4 * F0, (J0, CHI)))
                    V.tensor_sub(out=apx(a32t, dsti, (12, CHI)),
                                 in0=apx(tAh, 0, (1, CHI)),
                                 in1=apx(tBh, 0, (1, CHI)))

                # cumsums (slab-contiguous fp16)
                for t in range(1, T0):
                    V.tensor_add(
                        out=apx(w0, t * 3 * F0, (1, 3 * F0)),
                        in0=apx(w0, t * 3 * F0, (1, 3 * F0)),
                        in1=apx(w0, (t - 1) * 3 * F0, (1, 3 * F0)))
                    if t < T1:
                        V.tensor_add(
                            out=apx(w1, t * 3 * F1, (1, 3 * F1)),
                            in0=apx(w1, t * 3 * F1, (1, 3 * F1)),
                            in1=apx(w1, (t - 1) * 3 * F1, (1, 3 * F1)))

                # a32 translation (slab 0 of cumsum = w slab 0)
                V.tensor_copy(out=apx(a32t, 3, (12, CHI), (4, 3)),
                              in_=apx(w0, 32, (J0, CHI), (F0, 3)))

                # block-total HTs -> fp32 packed bht
                def emit_bht(X, w, tW1, bht, F, T):
                    base = (T - 1) * 9 * F
                    SC.copy(out=apx(bht, 0, (12, F), (4, 2), (1, 3)),
                            in_=apx(X, base, (1, F), (3 * F, 2), (F, 3)))
                    SC.copy(out=apx(bht, 8, (12, F)),
                            in_=apx(tW1, (T - 1) * F, (1, F)))
                    for dsti, (e1, e2), (e3, e4) in ((9, (2, 3), (0, 5)),
                                                     (10, (0, 4), (1, 3))):
                        V.tensor_mul(out=apx(tAh, 0, (1, F)),
                                     in0=apx(X, base + e1 * F, (1, F)),
                                     in1=apx(X, base + e2 * F, (1, F)))
                        V.tensor_mul(out=apx(tBh, 0, (1, F)),
                                     in0=apx(X, base + e3 * F, (1, F)),
                                     in1=apx(X, base + e4 * F, (1, F)))
                        V.tensor_sub(out=apx(bht, dsti, (12, F)),
                                     in0=apx(tAh, 0, (1, F)),
                                     in1=apx(tBh, 0, (1, F)))
                    SC.copy(out=apx(bht, 3, (12, F), (4, 3)),
                            in_=apx(w, (T - 1) * 3 * F, (1, F), (F, 3)))

                emit_bht(X0, w0, tW1_0, bht0, F0, T0)
                emit_bht(X1, w1, tW1_1, bht1, F1, T1)

                # ---- hierarchy (fp32, as v1) ----
                LPS0 = (U0 + 1) * 12
                LPS1 = (U1 + 1) * 12
                V.tensor_copy(out=apx(lp2_0, 12, (LPS0, CHI * S0), (1, 12)),
                              in_=apx(bht0, 0, (U0 * 12, CHI * S0), (1, 12)))
                SC.copy(out=apx(lp2_1, 12, (LPS1, CHI * S1), (1, 12)),
                        in_=apx(bht1, 0, (U1 * 12, CHI * S1), (1, 12)))
                for u in range(1, U0):
                    compose_1d(V, CHI * S0,
                               a_off=u * 12, a_step=LPS0,
                               b_off=u * 12, b_step=U0 * 12,
                               o_off=(u + 1) * 12, o_step=LPS0,
                               tA=tAh, tB=tBh,
                               a_tile=lp2_0, b_tile=bht0, o_tile=lp2_0)
                    if u < U1:
                        compose_1d(V, CHI * S1,
                                   a_off=u * 12, a_step=LPS1,
                                   b_off=u * 12, b_step=U1 * 12,
                                   o_off=(u + 1) * 12, o_step=LPS1,
                                   tA=tA1h, tB=tB1h,
                                   a_tile=lp2_1, b_tile=bht1, o_tile=lp2_1)
                for sidx in range(1, S0):
                    compose_1d(V, CHI,
                               a_off=(sidx - 1) * 12, a_step=S0 * 12,
                               b_off=(sidx - 1) * LPS0 + U0 * 12,
                               b_step=S0 * LPS0,
                               o_off=sidx * 12, o_step=S0 * 12,
                               tA=tAh, tB=tBh,
                               a_tile=spx0, b_tile=lp2_0, o_tile=spx0)
                excl_blocks(V, SC, CHI * S0, U0, LPS0, spx0, lp2_0, rx0,
                            tAh, tBh)
                compose_1d(V, CHI,
                           a_off=32 * 12, a_step=J0 * 12,
                           b_off=0, b_step=12,
                           o_off=0, o_step=12,
                           tA=tAh, tB=tBh,
                           a_tile=rx0, b_tile=a32t, o_tile=rbr)
                SC.copy(out=apx(spx1, 0, (S1 * 12, CHI), (1, 12)),
                        in_=apx(rbr, 0, (12, CHI), (1, 12)))
                # rx -> planar fp16 for the down transform
                V.tensor_copy(out=apx(rx16_0, 0, (F0, 12), (1, F0)),
                              in_=apx(rx0, 0, (1, 12), (12, F0)))

                def down_i(w, rx16, X, tmpoff, F, T, i):
                    xyz = apx(X, 0, (3 * F, T), (F, 3), (1, F))
                    tmp = apx(X, tmpoff, (3 * F, T), (F, 3), (1, F))

                    def rxk(k):
                        return apx(rx16, k * F, (0, T), (4 * F, 3), (1, F))

                    def wk(k):
                        return apx(w, k * F, (3 * F, T), (0, 3), (1, F))

                    if i == 0:
                        V.tensor_mul(out=xyz, in0=rxk(0), in1=wk(0))
                    elif i == 1:
                        V.tensor_mul(out=tmp, in0=rxk(1), in1=wk(1))
                    elif i == 2:
                        V.tensor_add(out=xyz, in0=xyz, in1=tmp)
                    elif i == 3:
                        V.tensor_mul(out=tmp, in0=rxk(2), in1=wk(2))
                    elif i == 4:
                        V.tensor_add(out=xyz, in0=xyz, in1=tmp)
                    else:
                        V.tensor_add(out=xyz, in0=xyz, in1=rxk(3))

                # gen1 level-3 + excl first (covers the rx16_0 cast on ACT),
                # then the down-transforms; xyz stays planar for the DMA and
                # the host undoes the layout.
                for sidx in range(1, S1):
                    compose_1d(V, CHI,
                               a_off=(sidx - 1) * 12, a_step=S1 * 12,
                               b_off=(sidx - 1) * LPS1 + U1 * 12,
                               b_step=S1 * LPS1,
                               o_off=sidx * 12, o_step=S1 * 12,
                               tA=tA1h, tB=tB1h,
                               a_tile=spx1, b_tile=lp2_1, o_tile=spx1)
                excl_blocks(V, SC, CHI * S1, U1, LPS1, spx1, lp2_1, rx1,
                            tA1h, tB1h)
                V.tensor_copy(out=apx(rx16_1, 0, (F1, 12), (1, F1)),
                              in_=apx(rx1, 0, (1, 12), (12, F1)))
                for i in range(6):
                    down_i(w0, rx16_0, X0, 3 * F0 * T0, F0, T0, i)
                nc.sync.dma_start(
                    out=AP(kin0_d, 0, [[F0 * T0 * 3, P], [1, F0 * T0 * 3]]),
                    in_=apx(X0, 0, (1, F0 * T0 * 3)))
                for i in range(6):
                    down_i(w1, rx16_1, X1, 3 * F1 * T1, F1, T1, i)
                nc.sync.dma_start(
                    out=AP(kin1_d, 0, [[F1 * T1 * 3, P], [1, F1 * T1 * 3]]),
                    in_=apx(X1, 0, (1, F1 * T1 * 3)))

    nc.compile()
    return nc


def get_program(repeat=1):
    key = ("nc", repeat)
    if key not in _CACHE:
        _CACHE[key] = _build_program(repeat)
    return _CACHE[key]


# ------------------------------------------------------------------- host
def _shard_inputs(dofs, doftype):
    """Per-core inputs, pre-transposed to device t-major lane order.

    Device order per partition p: index t*F + chi*J + j for atom
    (chi, j, t); host layout [P, CHI*L].  Alpha is pre-folded on the host:
    alpha_p = phi_c(parent) + phi_p(p) (chain starts: phi_p only; branch
    roots fold phi_c of gen0 atom 384)."""
    def to_dev(arr, J, T):
        # arr: [C_core, L] (chain-major) -> [P, T*CHI*J]
        a = arr.reshape(CHI, P, J, T)
        return np.ascontiguousarray(
            a.transpose(1, 3, 0, 2).reshape(P, CHI * J * T))

    chain_starts = 1 + np.arange(C0, dtype=np.int64) * L0
    jd_all = np.ascontiguousarray(dofs[chain_starts])       # [C0, 9]

    ph0 = dofs[1:BOFF, 0].reshape(C0, L0)
    th0 = dofs[1:BOFF, 1].reshape(C0, L0)
    d0 = dofs[1:BOFF, 2].reshape(C0, L0)
    pc0 = dofs[1:BOFF, 3].reshape(C0, L0)
    al0 = np.empty_like(ph0)
    al0[:, 0] = 0.0
    al0[:, 1] = ph0[:, 1]
    al0[:, 2:] = ph0[:, 2:] + pc0[:, 1:-1]

    ph1 = dofs[BOFF:, 0].reshape(C1, L1)
    th1 = dofs[BOFF:, 1].reshape(C1, L1)
    d1 = dofs[BOFF:, 2].reshape(C1, L1)
    pc1 = dofs[BOFF:, 3].reshape(C1, L1)
    al1 = np.empty_like(ph1)
    al1[:, 0] = ph1[:, 0] + pc0[:, 384]
    al1[:, 1:] = ph1[:, 1:] + pc1[:, :-1]

    in_maps = []
    for core in range(NCORES):
        s0 = slice(core * CH0, (core + 1) * CH0)
        s1 = slice(core * CH1, (core + 1) * CH1)
        jd = np.ascontiguousarray(
            jd_all[s0].reshape(CHI, P, 9).transpose(1, 0, 2)
            .reshape(P, CHI * 9))
        in_maps.append({
            "th0": to_dev(th0[s0], J0, T0),
            "al0": to_dev(al0[s0], J0, T0),
            "dt0": to_dev(d0[s0], J0, T0),
            "th1": to_dev(th1[s1], J1, T1),
            "al1": to_dev(al1[s1], J1, T1),
            "dt1": to_dev(d1[s1], J1, T1),
            "jd": jd,
        })
    return in_maps


def _lane_ids(id_idx, core):
    """id_idx values of this core's atoms in device lane order (p, f, t)."""
    ids0 = (id_idx[core * A0:(core + 1) * A0]
            .reshape(CHI, P, L0).transpose(1, 0, 2).ravel())
    ids1 = (id_idx[BOFF - 1 + core * A1: BOFF - 1 + (core + 1) * A1]
            .reshape(CHI, P, L1).transpose(1, 0, 2).ravel())
    return ids0, ids1


def _structure_ok(doftype, gen0_paths, gen1_paths):
    chain_starts = 1 + np.arange(C0, dtype=np.int64) * L0
    g0 = np.concatenate(
        [np.zeros((C0, 1), np.int64), chain_starts[:, None] + np.arange(L0)],
        axis=1)
    if not np.array_equal(gen0_paths, g0.astype(gen0_paths.dtype)):
        return False
    branch_roots = chain_starts + L0 // 2
    g1 = np.concatenate(
        [branch_roots[:, None],
         BOFF + (np.arange(C1, dtype=np.int64) * L1)[:, None] + np.arange(L1)],
        axis=1)
    if not np.array_equal(gen1_paths, g1.astype(gen1_paths.dtype)):
        return False
    if doftype[0] != 0:
        return False
    if not np.all(doftype[chain_starts] == 1):
        return False
    dt = doftype.copy()
    dt[chain_starts] = 2
    if not np.all(dt[1:] == 2):
        return False
    return True


def _numpy_fallback(dofs, doftype, gen0_paths, gen1_paths, id_idx):
    """Exact numpy port of the reference (slow path, safety net)."""
    def rx(a):
        c, s = np.cos(a), np.sin(a)
        o, z = np.ones_like(a), np.zeros_like(a)
        return np.stack([np.stack([o, z, z, z], -1), np.stack([z, c, -s, z], -1),
                         np.stack([z, s, c, z], -1), np.stack([z, z, z, o], -1)], -2)

    def ry(a):
        c, s = np.cos(a), np.sin(a)
        o, z = np.ones_like(a), np.zeros_like(a)
        return np.stack([np.stack([c, z, s, z], -1), np.stack([z, o, z, z], -1),
                         np.stack([-s, z, c, z], -1), np.stack([z, z, z, o], -1)], -2)

    def rz(a):
        c, s = np.cos(a), np.sin(a)
        o, z = np.ones_like(a), np.zeros_like(a)
        return np.stack([np.stack([c, -s, z, z], -1), np.stack([s, c, z, z], -1),
                         np.stack([z, z, o, z], -1), np.stack([z, z, z, o], -1)], -2)

    def trans(x, y, z):
        o, zr = np.ones_like(x), np.zeros_like(x)
        return np.stack([np.stack([o, zr, zr, x], -1), np.stack([zr, o, zr, y], -1),
                         np.stack([zr, zr, o, z], -1), np.stack([zr, zr, zr, o], -1)], -2)

    dofs = dofs.astype(np.float32)
    phi_p, theta, d, phi_c = dofs[:, 0], dofs[:, 1], dofs[:, 2], dofs[:, 3]
    z = np.zeros_like(d)
    bond = rx(phi_p) @ rz(np.pi - theta) @ trans(d, z, z) @ rx(phi_c)
    rot = lambda a, b, c: rz(c) @ ry(b) @ rx(a)
    jump = (trans(dofs[:, 0], dofs[:, 1], dofs[:, 2])
            @ rot(dofs[:, 3], dofs[:, 4], dofs[:, 5])
            @ rot(dofs[:, 6], dofs[:, 7], dofs[:, 8]))
    eye = np.broadcast_to(np.eye(4, dtype=dofs.dtype), bond.shape)
    dt = doftype[:, None, None]
    hts = np.where(dt == 1, jump, np.where(dt == 2, bond, eye)).astype(np.float32)
    for paths in (gen0_paths, gen1_paths):
        seg = hts[paths]
        out = np.empty_like(seg)
        out[:, 0] = seg[:, 0]
        for i in range(1, seg.shape[1]):
            out[:, i] = out[:, i - 1] @ seg[:, i]
        hts[paths] = out
    kincoords = hts[:, :3, 3]
    coords = np.zeros((N - 1, 3), dtype=dofs.dtype)
    coords[np.asarray(id_idx)] = kincoords[1:]
    return coords


def kernel(dofs, doftype, gen0_paths, gen1_paths, id_idx):
    dofs = np.asarray(dofs, dtype=np.float32)
    doftype = np.asarray(doftype, dtype=np.int32)
    gen0_paths = np.asarray(gen0_paths)
    gen1_paths = np.asarray(gen1_paths)
    id_idx = np.asarray(id_idx, dtype=np.int32)

    if not _structure_ok(doftype, gen0_paths, gen1_paths):
        return _numpy_fallback(dofs, doftype, gen0_paths, gen1_paths, id_idx)

    from concourse.bass_utils import run_bass_kernel_spmd

    nc = get_program()
    in_maps = _shard_inputs(dofs, doftype)
    res = run_bass_kernel_spmd(nc, in_maps, core_ids=list(range(NCORES)))
    out = np.empty((N - 1, 3), dtype=np.float32)
    for core in range(NCORES):
        ids0, ids1 = _lane_ids(id_idx, core)
        k0 = res.results[core]["kin0"].astype(np.float32)
        k0 = k0.reshape(P, T0, 3, F0).transpose(0, 3, 1, 2).reshape(-1, 3)
        k1 = res.results[core]["kin1"].astype(np.float32)
        k1 = k1.reshape(P, T1, 3, F1).transpose(0, 3, 1, 2).reshape(-1, 3)
        out[ids0] = k0
        out[ids1] = k1
    return out
